# revision 2
# baseline (speedup 1.0000x reference)
"""Trainium2 Bass kernel for nn_DilatedOCA — v5.

Math: see v3/v4 headers. v5 changes:
  - V^T tiles built by the DMA xbar transpose (16x128 tiles) + one strided
    SBUF->SBUF repack per quad — no PE transposes, no ones-row input
    (vt_all pre-memset to 1.0 supplies the softmax denominator column).
  - Engine streams planned around in-order execution: the Scalar engine
    (ACT) issues almost no DMAs so exp chunks start as soon as logits
    exist; k-quad-0 chain on sync unblocks QK ~40us in; v/k quads 1-3
    issue from gpsimd/scalar interleaved with the main loop.
  - PE at a fixed 1.2 GHz (this part never leaves HAM cold state):
    main-loop PE work = 3-way-tiled QK + 4-way-tiled PV only.
Sharding: 8 cores = 4 heads x 2 query-halves; host sums head partials.
"""

import sys

for _p in ("/opt/trn_rl_repo", "/root/.axon_site/_ro/pypackages"):
    if _p not in sys.path:
        sys.path.insert(0, _p)

import numpy as np
import ml_dtypes

import concourse.bass as bass
import concourse.mybir as mybir
import concourse.tile as tile
from concourse import bacc
from concourse.bass_utils import run_bass_kernel_spmd

F32 = mybir.dt.float32
BF16 = mybir.dt.bfloat16
I16 = mybir.dt.int16
AF = mybir.ActivationFunctionType
ALU = mybir.AluOpType

HEADS, DH = 4, 16
NPIX, NHALF = 4096, 2048
PADW = 68
PFREE = PADW * 16
M = 9216
MQ = 2304
NT = 72
NG = 24
EPS = 1e-5
LOG2E = 1.4426950408889634
A_SCH = 128.0 * LOG2E
B_SCH = 16256.0 - 5.62

_CACHE = {}


def _dve_chunk(g, nck):
    """Exp-engine split. The main loop is PE-paced, so ScalarE absorbs all
    exp chunks with ~3% runtime cost vs a 50/50 VectorE-Schraudolph split,
    while keeping rel_l2 at ~2.7e-3 instead of ~1.6e-2."""
    return False


def _build():
    nc = bacc.Bacc(trn_type="TRN2")
    x_d = nc.dram_tensor("x", [64, NPIX], F32, kind="ExternalInput")
    xq_d = nc.dram_tensor("xq", [64, NHALF], F32, kind="ExternalInput")
    statics_d = nc.dram_tensor("statics", [64, 49], F32, kind="ExternalInput")
    woutT_d = nc.dram_tensor("woutT", [16, 64], F32, kind="ExternalInput")
    ones1_d = nc.dram_tensor("ones1", [1, 64], F32, kind="ExternalInput")
    id128_d = nc.dram_tensor("id128", [128, 128], BF16, kind="ExternalInput")
    y_d = nc.dram_tensor("y", [64, NHALF], F32, kind="ExternalOutput")
    ktmp_d = nc.dram_tensor("ktmp", [NPIX, 16], BF16)
    vtmp_d = nc.dram_tensor("vtmp", [NPIX, 16], BF16)

    with tile.TileContext(nc) as tc:
        with tc.tile_pool(name="sb", bufs=1) as sb, \
             tc.tile_pool(name="sm", bufs=4) as sm, \
             tc.tile_pool(name="pp", bufs=8) as pp, \
             tc.tile_pool(name="ps", bufs=2, space="PSUM") as ps, \
             tc.tile_pool(name="pvp", bufs=1, space="PSUM") as pvp:
            xsb = sb.tile([64, NPIX], F32)
            xqsb = sb.tile([64, NHALF], F32)
            statics = sb.tile([64, 49], F32)
            woutT = sb.tile([16, 64], F32)
            ones1 = sb.tile([1, 64], F32)
            id128 = sb.tile([128, 128], BF16)
            xsq = sb.tile([64, NPIX], BF16)
            xsqq = sb.tile([64, NHALF], BF16)
            onescol = sb.tile([64, 1], BF16)
            stgraw = sb.tile([128, 50 * 32], F32)
            stgq = sb.tile([128, 18 * 16], F32)
            varsb = sb.tile([128, 48], F32)
            sdall = sb.tile([128, 48], F32)
            rstdT = sb.tile([128, 48], F32)
            stgkv = sb.tile([128, 1024], BF16)
            padk = sb.tile([PADW, PFREE], BF16)
            padv = sb.tile([PADW, PFREE], BF16)
            gkq = [sb.tile([16, MQ], BF16, name=f"gkq{i}") for i in range(4)]
            gvq = [sb.tile([16, MQ], BF16, name=f"gvq{i}") for i in range(4)]
            gk3q = [sb.tile([128, 6 * 128], BF16, name=f"gk3q{i}")
                    for i in range(4)]
            vtq = [sb.tile([128, 18 * 16], BF16, name=f"vtq{i}")
                   for i in range(4)]
            vt_all = sb.tile([128, 17 * NT], BF16)
            qsb = sb.tile([128, NHALF], BF16)
            osb = sb.tile([128, 512], F32)
            numb = sb.tile([16, NHALF], F32)
            denb = sb.tile([1, NHALF], F32)
            rbsb = sb.tile([64, NHALF], F32)
            ysb = sb.tile([64, NHALF], F32)
            touch = sb.tile([1, 16], BF16)

            pv = pvp.tile([128, 512], F32)

            for dst, src in ((xsb, x_d), (xqsb, xq_d), (statics, statics_d),
                             (woutT, woutT_d), (ones1, ones1_d),
                             (id128, id128_d)):
                nc.sync.dma_start(out=dst[:, :], in_=src[:, :])
            nc.gpsimd.memset(padk[:, :], 0.0)
            nc.gpsimd.memset(padv[:, :], 0.0)
            nc.gpsimd.memset(onescol[:, :], 1.0)
            nc.gpsimd.memset(vt_all[:, :], 1.0)

            # warmup matmuls: absorb DMA-queue sems one at a time (walrus
            # allows only 1 sync wait per Matmult).
            wt = ps.tile([128, 130], F32, tag="st")
            nc.tensor.matmul(wt[:, 0:128], xsb[:, 0:128],
                             xsb[:, 0:128], start=True, stop=True)
            nc.tensor.matmul(wt[0:49, 128:129], statics[:, :],
                             statics[:, 0:1], start=True, stop=True)
            nc.tensor.matmul(wt[:, 129:130], xqsb[:, 0:128],
                             xqsb[:, 0:1], start=True, stop=True)
            wtb = ps.tile([128, 128], BF16, tag="st")
            nc.tensor.transpose(wtb[:, :], id128[:, :], id128[:, :])

            nc.vector.tensor_tensor(out=xsqq[:, :], in0=xqsb[:, :],
                                    in1=xqsb[:, :], op=ALU.mult)
            for c in range(4):
                cs = slice(1024 * c, 1024 * (c + 1))
                nc.vector.tensor_tensor(out=xsq[:, cs], in0=xsb[:, cs],
                                        in1=xsb[:, cs], op=ALU.mult)

            # ---- q chunks: 32-col bank-aligned slots, one strided copy --
            ps2 = ps.tile([128, 512], F32, tag="st")
            for u in range(16):
                xc = xqsb[:, 128 * u:128 * (u + 1)]
                nc.tensor.matmul(ps2[:, 32 * u:32 * u + 17], xc,
                                 statics[:, 32:49], start=True, stop=True)
                nc.tensor.matmul(ps2[:, 32 * u + 17:32 * u + 18],
                                 xsqq[:, 128 * u:128 * (u + 1)],
                                 onescol[:, :], start=True, stop=True)
            src_ap = bass.AP(tensor=ps2.tensor, offset=0,
                             ap=[[512, 128], [32, 16], [1, 18]])
            dst_ap = bass.AP(tensor=stgq.tensor, offset=0,
                             ap=[[18 * 16, 128], [18, 16], [1, 18]])
            nc.vector.tensor_copy(dst_ap, src_ap)

            s1q = bass.AP(tensor=stgq.tensor, offset=16,
                          ap=[[18 * 16, 128], [18, 16]])
            s2q = bass.AP(tensor=stgq.tensor, offset=17,
                          ap=[[18 * 16, 128], [18, 16]])
            tmp = sm.tile([128, 48], F32, tag="t0")
            nc.vector.tensor_tensor(out=tmp[:, 32:48], in0=s1q, in1=s1q,
                                    op=ALU.mult)
            nc.vector.scalar_tensor_tensor(
                out=varsb[:, 32:48], in0=tmp[:, 32:48], scalar=-1.0 / 64,
                in1=s2q, op0=ALU.mult, op1=ALU.add)
            nc.vector.tensor_scalar(
                out=varsb[:, 32:48], in0=varsb[:, 32:48],
                scalar1=1.0 / 64, scalar2=EPS, op0=ALU.mult, op1=ALU.add)
            nc.scalar.activation(sdall[:, 32:48], varsb[:, 32:48], AF.Sqrt)
            nc.vector.reciprocal_approx_fast(out=rstdT[:, 32:48],
                                             in_=sdall[:, 32:48])

            def q_stage(u):
                qstage = sm.tile([128, 16], BF16, tag="qs")
                nc.vector.tensor_scalar(
                    out=qstage[:, :], in0=stgq[:, 18 * u:18 * u + 16],
                    scalar1=rstdT[:, 32 + u:33 + u], scalar2=None,
                    op0=ALU.mult)
                qtp = ps.tile([16, 128], BF16, tag="st")
                nc.tensor.transpose(qtp[:, :], qstage[:, :], id128[:, :])
                nc.vector.tensor_copy(qsb[0:16, 128 * u:128 * (u + 1)],
                                      qtp[:, :])

            # ---- kv chunks: 64-col bank-aligned slots (24 + 8) ---------
            for blk, n in ((0, 24), (24, 8)):
                ps1 = ps.tile([128, 1536], F32, tag="st")
                for tt in range(n):
                    t = blk + tt
                    xc = xsb[:, 128 * t:128 * (t + 1)]
                    nc.tensor.matmul(ps1[:, 64 * tt:64 * tt + 49], xc,
                                     statics[:, :], start=True, stop=True)
                    nc.tensor.matmul(ps1[:, 64 * tt + 49:64 * tt + 50],
                                     xsq[:, 128 * t:128 * (t + 1)],
                                     onescol[:, :], start=True, stop=True)
                src_ap = bass.AP(tensor=ps1.tensor, offset=0,
                                 ap=[[1536, 128], [64, n], [1, 50]])
                dst_ap = bass.AP(tensor=stgraw.tensor, offset=50 * blk,
                                 ap=[[50 * 32, 128], [50, n], [1, 50]])
                nc.vector.tensor_copy(dst_ap, src_ap)
            for u in range(16):
                q_stage(u)
            for i in (32, 64):
                src_ap = bass.AP(tensor=qsb.tensor, offset=0,
                                 ap=[[NHALF, 16], [1, NHALF]])
                dst_ap = bass.AP(tensor=qsb.tensor, offset=i * NHALF,
                                 ap=[[NHALF, 16], [1, NHALF]])
                nc.gpsimd.dma_start(out=dst_ap, in_=src_ap)

            s1kv = bass.AP(tensor=stgraw.tensor, offset=48,
                           ap=[[50 * 32, 128], [50, 32]])
            s2kv = bass.AP(tensor=stgraw.tensor, offset=49,
                           ap=[[50 * 32, 128], [50, 32]])
            nc.vector.tensor_tensor(out=tmp[:, 0:32], in0=s1kv, in1=s1kv,
                                    op=ALU.mult)
            nc.vector.scalar_tensor_tensor(
                out=varsb[:, 0:32], in0=tmp[:, 0:32], scalar=-1.0 / 64,
                in1=s2kv, op0=ALU.mult, op1=ALU.add)
            nc.vector.tensor_scalar(
                out=varsb[:, 0:32], in0=varsb[:, 0:32],
                scalar1=1.0 / 64, scalar2=EPS, op0=ALU.mult, op1=ALU.add)
            nc.scalar.activation(sdall[:, 0:32], varsb[:, 0:32], AF.Sqrt)
            nc.vector.reciprocal_approx_fast(out=rstdT[:, 0:32],
                                             in_=sdall[:, 0:32])

            for t in range(32):
                nc.vector.tensor_scalar(
                    out=stgkv[:, 32 * t:32 * (t + 1)],
                    in0=stgraw[:, 50 * t:50 * t + 32],
                    scalar1=rstdT[:, t:t + 1], scalar2=None, op0=ALU.mult)

            # ---- staging -> DRAM -> padded images (sync) ---------------
            for tmp_d, c0 in ((ktmp_d, 0), (vtmp_d, 16)):
                src_ap = bass.AP(tensor=stgkv.tensor, offset=c0,
                                 ap=[[1024, 128], [32, 32], [1, 16]])
                dst_ap = bass.AP(tensor=tmp_d, offset=0,
                                 ap=[[16, 128], [2048, 32], [1, 16]])
                nc.sync.dma_start(out=dst_ap, in_=src_ap)
            for tmp_d, pad_t in ((ktmp_d, padk), (vtmp_d, padv)):
                src_ap = bass.AP(tensor=tmp_d, offset=0,
                                 ap=[[1024, 64], [1, 1024]])
                dst_ap = bass.AP(tensor=pad_t.tensor,
                                 offset=2 * PFREE + 2 * 16,
                                 ap=[[PFREE, 64], [1, 1024]])
                nc.sync.dma_start(out=dst_ap, in_=src_ap)

            def gather_quad(khH, pad_t, g_t, eng):
                for khL in range(3):
                    for lq in range(4):
                        row0 = 16 * lq + 3 * khH + khL
                        src_ap = bass.AP(
                            tensor=pad_t.tensor, offset=row0 * PFREE,
                            ap=[[8 * PFREE, 2], [128, 8], [1, 192]])
                        dst_ap = bass.AP(
                            tensor=g_t.tensor, offset=768 * khL + 192 * lq,
                            ap=[[MQ, 16], [1, 192]])
                        eng.dma_start(out=dst_ap, in_=src_ap)

            def redistribute(q, eng):
                for i in range(3):
                    src_ap = bass.AP(tensor=gkq[q].tensor, offset=128 * i,
                                     ap=[[MQ, 16], [384, 6], [1, 128]])
                    dst_ap = bass.AP(tensor=gk3q[q].tensor,
                                     offset=32 * i * 768,
                                     ap=[[768, 16], [128, 6], [1, 128]])
                    eng.dma_start(out=dst_ap, in_=src_ap)

            def vt_build(q, eng):
                # xbar transpose to contiguous vtq, then strided repack
                # into vt_all (ones column at 17T+16 pre-set by memset)
                out_ap = bass.AP(tensor=vtq[q].tensor, offset=0,
                                 ap=[[288, 128], [16, 18], [1, 16]])
                eng.dma_start_transpose(out=out_ap, in_=gvq[q][:, :])
                src_ap = bass.AP(tensor=vtq[q].tensor, offset=0,
                                 ap=[[288, 128], [16, 18], [1, 16]])
                dst_ap = bass.AP(tensor=vt_all.tensor, offset=17 * 18 * q,
                                 ap=[[17 * NT, 128], [17, 18], [1, 16]])
                eng.dma_start(out=dst_ap, in_=src_ap)

            # sync: k q0 chain first (unblocks QK), then v q0 + vt, k q1
            gather_quad(0, padk, gkq[0], nc.sync)
            redistribute(0, nc.sync)
            gather_quad(0, padv, gvq[0], nc.sync)
            vt_build(0, nc.sync)
            gather_quad(1, padk, gkq[1], nc.sync)
            redistribute(1, nc.sync)
            # gpsimd: v q1-3 then k q2-3
            for q in (1, 2, 3):
                gather_quad(q, padv, gvq[q], nc.gpsimd)
            for q in (2, 3):
                gather_quad(q, padk, gkq[q], nc.gpsimd)
                redistribute(q, nc.gpsimd)

            # DVE touches: absorb DMA-queue sems so matmuls keep <=1 wait
            nc.vector.tensor_copy(touch[0:1, 0:1], qsb[0:1, 0:1])
            nc.vector.tensor_copy(touch[0:1, 1:2], vt_all[0:1, 0:1])
            nc.vector.tensor_copy(touch[0:1, 2:3], gk3q[0][0:1, 0:1])

            # ================= main loop =================
            def qk1(g, nck):
                gq, gl = g // 6, g % 6
                ncs = slice(512 * nck, 512 * (nck + 1))
                st = ps.tile([128, 1536], F32, tag="st")
                for i in range(3):
                    nc.tensor.matmul(
                        st[:, 512 * i:512 * (i + 1)],
                        gk3q[gq][32 * i:32 * i + 16,
                                 128 * gl:128 * (gl + 1)],
                        qsb[32 * i:32 * i + 16, ncs],
                        start=True, stop=True)
                pt = pp.tile([128, 1536], BF16, tag="pt")
                if _dve_chunk(g, nck):
                    nc.vector.tensor_scalar(
                        out=pt[:, :].bitcast(I16), in0=st[:, :],
                        scalar1=A_SCH, scalar2=B_SCH,
                        op0=ALU.mult, op1=ALU.add)
                else:
                    nc.scalar.activation(pt[:, :], st[:, :], AF.Exp)
                return pt

            def pv1(g, i, pts):
                T = 3 * g + i
                for nck in range(4):
                    nc.tensor.matmul(
                        pv[32 * nck:32 * nck + 17, :],
                        vt_all[:, 17 * T:17 * (T + 1)],
                        pts[nck][:, 512 * i:512 * (i + 1)],
                        start=(T == 0), stop=(T == NT - 1),
                        tile_position=(0, 32 * nck))

            prev = None
            for g in range(NG):
                # scalar-engine vt builds for quads 1-3 at safe points
                if g in (2, 6, 10):
                    vt_build(1 + (g - 2) // 4, nc.scalar)
                # late touches for later k quads (absorb queue sems)
                if g in (4, 10, 16):
                    qn = g // 6 + 1
                    nc.vector.tensor_copy(touch[0:1, 3 + qn:4 + qn],
                                          gk3q[qn][0:1, 0:1])
                pts = []
                pts.append(qk1(g, 0))
                pts.append(qk1(g, 1))
                if prev is not None:
                    pv1(g - 1, 0, prev)
                    pv1(g - 1, 1, prev)
                pts.append(qk1(g, 2))
                if prev is not None:
                    pv1(g - 1, 2, prev)
                pts.append(qk1(g, 3))
                prev = pts
            for i in range(3):
                pv1(NG - 1, i, prev)

            nc.vector.tensor_copy(osb[:, :], pv[:, :])

            # ================= tail =================
            for nck in range(4):
                nc.sync.dma_start(
                    out=bass.AP(tensor=numb.tensor, offset=512 * nck,
                                ap=[[NHALF, 16], [1, 512]]),
                    in_=bass.AP(tensor=osb.tensor, offset=32 * nck * 512,
                                ap=[[512, 16], [1, 512]]))
                nc.scalar.dma_start(
                    out=bass.AP(tensor=denb.tensor, offset=512 * nck,
                                ap=[[NHALF, 1], [1, 512]]),
                    in_=bass.AP(tensor=osb.tensor,
                                offset=(32 * nck + 16) * 512,
                                ap=[[512, 1], [1, 512]]))
            for h in range(2):
                hs = slice(1024 * h, 1024 * (h + 1))
                yp = ps.tile([64, 1024], F32, tag="st")
                for j in range(2):
                    ncs = slice(1024 * h + 512 * j, 1024 * h + 512 * (j + 1))
                    nc.tensor.matmul(yp[:, 512 * j:512 * (j + 1)],
                                     woutT[:, :], numb[:, ncs],
                                     start=True, stop=True)
                bp = ps.tile([64, 1024], F32, tag="st")
                for j in range(2):
                    ncs = slice(1024 * h + 512 * j, 1024 * h + 512 * (j + 1))
                    nc.tensor.matmul(bp[:, 512 * j:512 * (j + 1)],
                                     ones1[:, :], denb[:, ncs],
                                     start=True, stop=True)
                nc.vector.reciprocal_approx_fast(out=rbsb[:, hs],
                                                 in_=bp[:, :])
                nc.vector.tensor_tensor(out=ysb[:, hs], in0=yp[:, :],
                                        in1=rbsb[:, hs], op=ALU.mult)
            nc.sync.dma_start(out=y_d[:, :], in_=ysb[:, :])

    nc.compile()
    return nc


def _get_nc():
    if "nc" not in _CACHE:
        _CACHE["nc"] = _build()
    return _CACHE["nc"]


def kernel(x, w_qkv, w_out, ln_w, _want_trace=False):
    x = np.asarray(x, np.float32)
    w_qkv = np.asarray(w_qkv, np.float32)
    w_out = np.asarray(w_out, np.float32)
    ln_w = np.asarray(ln_w, np.float32)

    x2d = np.ascontiguousarray(x.reshape(64, NPIX))
    ones1 = np.ones((1, 64), np.float32)
    id128 = np.eye(128).astype(ml_dtypes.bfloat16)
    lw = ln_w[None, :]

    in_maps = []
    for c in range(8):
        h, half = c % 4, c // 4
        wk = w_qkv[64 + 16 * h:64 + 16 * h + 16, :] * lw
        wv = w_qkv[128 + 16 * h:128 + 16 * h + 16, :] * lw
        wq = 0.25 * w_qkv[16 * h:16 * h + 16, :] * lw
        statics = np.concatenate(
            [wk.T, wv.T, wq.T, np.ones((64, 1), np.float32)],
            axis=1).astype(np.float32)
        in_maps.append({
            "x": x2d,
            "xq": np.ascontiguousarray(x2d[:, NHALF * half:NHALF * (half + 1)]),
            "statics": np.ascontiguousarray(statics),
            "woutT": np.ascontiguousarray(
                w_out[:, 16 * h:16 * h + 16].T.astype(np.float32)),
            "ones1": ones1,
            "id128": id128,
        })

    nc = _get_nc()
    res = run_bass_kernel_spmd(nc, in_maps, list(range(8)), trace=_want_trace)
    if _want_trace:
        _CACHE["last_result"] = res

    y = np.empty((64, NPIX), np.float32)
    for half in range(2):
        acc = np.zeros((64, NHALF), np.float32)
        for h in range(4):
            acc += res.results[4 * half + h]["y"]
        y[:, NHALF * half:NHALF * (half + 1)] = acc
    return y.reshape(1, 64, 64, 64)
